# revision 12
# baseline (speedup 1.0000x reference)
"""MoE routed dense layer (nn_MultiHeadDense): y[b] = x[b] @ W[idx[b]] + bias[idx[b]].

Full shapes: inputs [4096,1024] f32, indices [4096] int, kernel [8,1024,1024] f32,
bias [8,1024] f32 -> out [4096,1024] f32.

Sharding strategy (expert-parallel, H == n_cores == 8): core h owns expert h's
weight [1024,1024] and processes up to C=512 of the rows routed to expert h.
The host computes the per-expert row lists from `indices`, gathers each
expert's first C rows into a zero-padded transposed activation block
XT_h [D, C], and scatters the per-core outputs back into the full [B, F]
result. Rows beyond C on an overloaded expert (~1% of rows for balanced
routing) are computed on the host in f32; this keeps the device at exactly
4 full 128-row m-tiles (64 matmuls) instead of 5 mostly-empty ones.

On-device per core: Y[c, f] = sum_k XT[k*128:(k+1)*128, c].T @ W[k*128:.., f]
accumulated in PSUM over the 8 k-tiles. The bias is added on the host during
the scatter (exact f32), so the device eviction is a plain PSUM->SBUF copy.
X and W are pre-cast to fp16 on the host (11-bit mantissa keeps the absmax
error ~1e-3 of output scale while halving HBM traffic and enabling the fast
PE weight-load path); accumulation stays fp32 in PSUM.

Schedule: the W+X stream arrives as 6 chunks on the sync HWDGE ring. Phase 1
runs k0..k3 for all four m-tiles k-outermost, racing the DMA fill; phase 2
finishes k4..k7 one (m, n) PSUM-bank half at a time, evicting each 256 KB
half (copy + DMA, engines/rings alternating scalar/vector) as soon as its
accumulation completes, so the output overlaps the remaining matmuls and the
last half leaves right after the final matmul. Zero-matmul warmup bridges PE
activity from queue start until chunk 0 lands so the HAM full-duty window is
granted right as the real stream starts; trailing zero-matmuls keep the PE
clock un-gated (K=8/8) through the backend's end-of-program event-reset walk,
which dispatches on the PE clock and would otherwise run at half rate.
"""

from contextlib import ExitStack

import numpy as np

import concourse.bass as bass
import concourse.tile as tile
from concourse import bacc, mybir
from concourse.bass_utils import run_bass_kernel_spmd

F32 = mybir.dt.float32
F16 = mybir.dt.float16

P = 128          # SBUF partitions / matmul tile edge
NTILE = 512      # matmul moving free dim (one fp32 PSUM bank)
CAP = 512        # device rows per core; overflow rows computed on host
WARMUP_MM = 5    # zero-matmuls bridging PE idle until chunk 0 lands
COOLDOWN_MM = 6  # zero-matmuls holding K=8/8 through the epilogue walk
KCHUNKS = (1, 1, 1, 1, 2, 2)   # k-tiles per input-stream chunk
PHASE1_K = 4     # k0..PHASE1_K-1 run k-outer; the rest run m-outer + evict


def _build(nc: bass.Bass, C: int, D: int, F: int,
           warmup=WARMUP_MM, cooldown=COOLDOWN_MM):
    KT = D // P
    NT = F // NTILE
    MT = C // P
    assert C % P == 0 and sum(KCHUNKS) == KT
    Q = F + C        # columns per k-tile in the fused stream

    wx = nc.dram_tensor("wx", (KT * P * Q,), F16, kind="ExternalInput").ap()
    y = nc.dram_tensor("y", (C, F), F32, kind="ExternalOutput").ap()

    with tile.TileContext(nc) as tc, ExitStack() as ctx:
        cp = ctx.enter_context(tc.tile_pool(name="cp", bufs=1))
        zp = ctx.enter_context(tc.tile_pool(name="zp", bufs=1))
        pp = ctx.enter_context(tc.tile_pool(name="pp", bufs=4, space="PSUM"))
        yp = ctx.enter_context(tc.tile_pool(name="yp", bufs=8))

        # Input stream (W+X chunks) on the sync HWDGE ring.
        wx_c = []
        off = 0
        for c, kg in enumerate(KCHUNKS):
            q = kg * Q
            ct = cp.tile([P, q], F16, name=f"wx{c}", tag=f"wx{c}")
            nc.sync.dma_start(
                ct[:], wx[off:off + P * q].rearrange("(p q) -> p q", p=P))
            wx_c.append(ct)
            off += P * q

        ps = [pp.tile([P, F], F32, name=f"ps{m}", tag="ps") for m in range(MT)]

        # PE warmup: zero matmuls (no DMA dependency) keep the PE busy
        # until chunk 0's completion receipt lands, so the HAM clock-gate
        # warmup (sustained activity before the PE runs at 2.4 GHz)
        # overlaps the DMA fill instead of following it. They target
        # ps[0], which the first real k=0 matmul resets via start=True.
        zt = zp.tile([P, NTILE], F16)
        nc.vector.memset(zt[:], 0.0)
        for _ in range(warmup):
            nc.tensor.matmul(ps[0][:, :NTILE], lhsT=zt[:, :P], rhs=zt[:],
                             start=True, stop=True)

        kmap = []  # k -> (chunk, index within chunk)
        for c, kg in enumerate(KCHUNKS):
            kmap.extend((c, ki) for ki in range(kg))

        def mm(m, k, n):
            c, ki = kmap[k]
            t = wx_c[c]
            xbase = ki * Q + F
            wbase = ki * Q + n * NTILE
            nc.tensor.matmul(
                ps[m][:, n * NTILE:(n + 1) * NTILE],
                lhsT=t[:, xbase + m * P:xbase + (m + 1) * P],
                rhs=t[:, wbase:wbase + NTILE],
                start=(k == 0),
                stop=(k == KT - 1),
            )

        for k in range(PHASE1_K):
            for m in range(MT):
                for n in range(NT):
                    mm(m, k, n)
        # Phase 2: finish each m-tile (k-inner so the n0/n1 matmul pairs
        # keep a shared lhsT for the LDWEIGHTS dedup) and evict its two
        # PSUM-bank halves; the n0 copy only waits on the k7/n0 matmul so
        # it overlaps k7/n1. Copy engines and DMA rings alternate
        # scalar/gpsimd+vector so consecutive 256 KB output DMAs overlap.
        for m in range(MT):
            for k in range(PHASE1_K, KT):
                for n in range(NT):
                    mm(m, k, n)
            for n in range(NT):
                i = m * NT + n
                yt = yp.tile([P, NTILE], F32, name=f"yt{i}", tag=f"y{i}")
                src = ps[m][:, n * NTILE:(n + 1) * NTILE]
                if i % 2 == 0:
                    nc.scalar.copy(yt[:], src)
                    nc.scalar.dma_start(
                        y[m * P:(m + 1) * P, n * NTILE:(n + 1) * NTILE], yt[:])
                else:
                    nc.vector.tensor_copy(yt[:], src)
                    nc.gpsimd.dma_start(
                        y[m * P:(m + 1) * P, n * NTILE:(n + 1) * NTILE], yt[:])

        # Cooldown: keep the PE array active past the last real matmul so
        # the HAM clock gate stays at K=8/8 while the output drains and
        # the epilogue event-reset walk (dispatched at the PE clock) runs.
        for _ in range(cooldown):
            nc.tensor.matmul(ps[0][:, :NTILE], lhsT=zt[:, :P], rhs=zt[:],
                             start=True, stop=True)


def _dedup_ldweights(nc):
    """Drop InstLdweights whose stationary operand is identical to the PE
    array's current contents (loaded by the previous InstLdweights with only
    matmuls in between). The PE array state is persistent, so the paired
    matmul reuses the already-loaded weights; this halves the PE instruction
    count for n0/n1 matmul pairs sharing an lhsT and for the warmup/cooldown
    runs, shortening both the stream and the epilogue event-reset walk."""
    def key(i):
        a = i.ins[0]
        return (str(a.memref), a.offset, str(a.ap), str(a.dtype),
                i.perf_mode, i.is_transpose, i.tile_position)

    for blk in nc.main_func.blocks:
        out = []
        last = None
        for i in blk.instructions:
            if getattr(i, "engine", None) != mybir.EngineType.PE:
                out.append(i)
                continue
            if isinstance(i, mybir.InstLdweights):
                si = i.sync_info
                clean = si is None or (not si.on_wait and not si.on_update)
                if clean and last is not None and key(i) == last:
                    continue
                last = key(i) if clean else None
                out.append(i)
            elif isinstance(i, mybir.InstMatmult):
                out.append(i)
            else:
                last = None
                out.append(i)
        blk.instructions[:] = out


LAST_PROFILE = {}


def kernel(inputs, indices, kernel, bias, _trace=False):
    x = np.ascontiguousarray(np.asarray(inputs), dtype=np.float32)
    idx = np.asarray(indices).astype(np.int64)
    wk = np.asarray(kernel, dtype=np.float32)
    bv = np.asarray(bias, dtype=np.float32)

    B, D = x.shape
    H, _, F = wk.shape
    C = CAP

    rows = [np.nonzero(idx == h)[0] for h in range(H)]
    kept = [r[:C] for r in rows]
    over = [r[C:] for r in rows]

    def pack(w16, xt16):
        # fused stream: per k-chunk one [P, kg*(F+C)] block where
        # block[p, ki*(F+C) + 0:F]   = W[(k0+ki)*P + p, :]
        # block[p, ki*(F+C) + F:F+C] = XT[(k0+ki)*P + p, :]
        KTl = w16.shape[0] // P
        fused = np.concatenate(
            [w16.reshape(KTl, P, F), xt16.reshape(KTl, P, C)], axis=2
        )  # [KT, P, F+C]
        parts = []
        k0 = 0
        for kg in KCHUNKS:
            blk = fused[k0:k0 + kg]  # [kg, P, Q]
            parts.append(blk.transpose(1, 0, 2).reshape(-1))
            k0 += kg
        return np.concatenate(parts)

    in_maps = []
    for h in range(H):
        r = kept[h]
        xt = np.zeros((D, C), dtype=np.float16)
        xt[:, :len(r)] = x[r].T
        in_maps.append({"wx": pack(wk[h].astype(np.float16), xt)})

    nc = bacc.Bacc(
        "TRN2", target_bir_lowering=False, debug=False, num_devices=H,
        enable_asserts=False,
    )
    _build(nc, C, D, F)
    _dedup_ldweights(nc)
    nc.compile()

    trace_kwargs = (
        {"trace": True, "trace_cores": list(range(H)), "stitch_traces": False}
        if _trace
        else {}
    )
    res = run_bass_kernel_spmd(nc, in_maps, core_ids=list(range(H)), **trace_kwargs)
    if _trace:
        LAST_PROFILE.clear()
        LAST_PROFILE.update(
            exec_time_ns=res.exec_time_ns,
            mean_exec_time_ns=res.mean_exec_time_ns,
            max_exec_time_core_id=res.max_exec_time_core_id,
            trace=res.instructions_and_trace[1] if res.instructions_and_trace else None,
            profile_json=res.profile_json,
        )

    out = np.empty((B, F), dtype=np.float32)
    for h in range(H):
        r = kept[h]
        out[r] = res.results[h]["y"][:len(r)] + bv[h]
        if len(over[h]):
            out[over[h]] = x[over[h]] @ wk[h] + bv[h]
    return out


# revision 20
# speedup vs baseline: 1.0060x; 1.0060x over previous
"""MoE routed dense layer (nn_MultiHeadDense): y[b] = x[b] @ W[idx[b]] + bias[idx[b]].

Full shapes: inputs [4096,1024] f32, indices [4096] int, kernel [8,1024,1024] f32,
bias [8,1024] f32 -> out [4096,1024] f32.

Sharding strategy (expert-parallel, H == n_cores == 8): core h owns expert h's
weight [1024,1024] and processes up to C=512 of the rows routed to expert h.
The host computes the per-expert row lists from `indices`, gathers each
expert's first C rows into a zero-padded transposed activation block
XT_h [D, C], and scatters the per-core outputs back into the full [B, F]
result. Rows beyond C on an overloaded expert (~1% of rows for balanced
routing) are computed on the host in f32; this keeps the device at exactly
4 full 128-row m-tiles (64 matmuls) instead of 5 mostly-empty ones.

On-device per core: Y[c, f] = sum_k XT[k*128:(k+1)*128, c].T @ W[k*128:.., f]
accumulated in PSUM over the 8 k-tiles. The bias is added on the host during
the scatter (exact f32), so the device eviction is a plain PSUM->SBUF copy.
X and W are pre-cast to fp16 on the host (11-bit mantissa keeps the absmax
error ~1e-3 of output scale while halving HBM traffic and enabling the fast
PE weight-load path); accumulation stays fp32 in PSUM.

Schedule: the W+X stream arrives as 6 chunks on the sync HWDGE ring. Phase 1
runs k0..k3 for all four m-tiles k-outermost, racing the DMA fill; phase 2
finishes k4..k7 one (m, n) PSUM-bank half at a time, evicting each 256 KB
half (copy + DMA, engines/rings alternating scalar/vector) as soon as its
accumulation completes, so the output overlaps the remaining matmuls and the
last half leaves right after the final matmul. Zero-matmul warmup bridges PE
activity from queue start until chunk 0 lands so the HAM full-duty window is
granted right as the real stream starts; trailing zero-matmuls keep the PE
clock un-gated (K=8/8) through the backend's end-of-program event-reset walk,
which dispatches on the PE clock and would otherwise run at half rate.
"""

from contextlib import ExitStack

import numpy as np

import concourse.bass as bass
import concourse.tile as tile
from concourse import bacc, mybir
from concourse.bass_utils import run_bass_kernel_spmd

F32 = mybir.dt.float32
F16 = mybir.dt.float16

P = 128          # SBUF partitions / matmul tile edge
NTILE = 512      # matmul moving free dim (one fp32 PSUM bank)
CAP = 512        # device rows per core; overflow rows computed on host
WARMUP_MM = 6    # zero-matmuls bridging PE idle until chunk 0a lands
COOLDOWN_MM = 9  # zero-matmuls holding K=8/8 through the output drain
KCHUNKS = (1, 1, 1, 1, 2, 2)   # k-tiles per input-stream chunk
PHASE1_K = 5     # k0..PHASE1_K-1 run k-outer; the rest run m-outer + evict


def _build(nc: bass.Bass, C: int, D: int, F: int,
           warmup=WARMUP_MM, cooldown=COOLDOWN_MM):
    KT = D // P
    NT = F // NTILE
    MT = C // P
    assert C % P == 0 and sum(KCHUNKS) == KT
    Q = F + C        # columns per k-tile in the fused stream

    wx = nc.dram_tensor("wx", (KT * P * Q,), F16, kind="ExternalInput").ap()
    y = nc.dram_tensor("y", (C, F), F32, kind="ExternalOutput").ap()
    scratch = nc.dram_tensor("scratch", (P, 16), F16, kind="ExternalOutput").ap()

    with tile.TileContext(nc) as tc, ExitStack() as ctx:
        cp = ctx.enter_context(tc.tile_pool(name="cp", bufs=1))
        zp = ctx.enter_context(tc.tile_pool(name="zp", bufs=1))
        pp = ctx.enter_context(tc.tile_pool(name="pp", bufs=4, space="PSUM"))
        yp = ctx.enter_context(tc.tile_pool(name="yp", bufs=8))

        # Input stream (W+X chunks) on the sync HWDGE ring. Chunk 0 is
        # column-ordered [X_k0 | W_k0] and delivered as two DMAs so the
        # k0/n0 matmuls (the first real work) gate on only the leading
        # [X | W_n0] 2/3 of the chunk.
        wx_c = []
        off = 0
        for c, kg in enumerate(KCHUNKS):
            q = kg * Q
            ct = cp.tile([P, q], F16, name=f"wx{c}", tag=f"wx{c}")
            src = wx[off:off + P * q].rearrange("(p q) -> p q", p=P)
            if c == 0 and kg == 1:
                split = C + NTILE
                nc.sync.dma_start(ct[:, :split], src[:, :split])
                nc.sync.dma_start(ct[:, split:], src[:, split:])
            else:
                nc.sync.dma_start(ct[:], src)
            wx_c.append(ct)
            off += P * q

        ps = [pp.tile([P, F], F32, name=f"ps{m}", tag="ps") for m in range(MT)]

        # PE warmup: zero matmuls (no DMA dependency) keep the PE busy
        # until chunk 0's completion receipt lands, so the HAM clock-gate
        # warmup (sustained activity before the PE runs at 2.4 GHz)
        # overlaps the DMA fill instead of following it. They target
        # ps[0], which the first real k=0 matmul resets via start=True.
        zt = zp.tile([P, NTILE], F16)
        nc.vector.memset(zt[:], 0.0)
        # Warm the two output DMA rings (scalar, gpsimd) with a tiny
        # transfer each so the evictions later don't pay the ~0.8 us
        # first-DMA ring-startup latency on the kernel's critical tail.
        nc.scalar.dma_start(scratch[:, :8], zt[:, :8])
        nc.gpsimd.dma_start(scratch[:, 8:], zt[:, 8:16])
        for _ in range(warmup):
            nc.tensor.matmul(ps[0][:, :NTILE], lhsT=zt[:, :P], rhs=zt[:],
                             start=True, stop=True)

        kmap = []  # k -> (chunk, index within chunk)
        for c, kg in enumerate(KCHUNKS):
            kmap.extend((c, ki) for ki in range(kg))

        def mm(m, k, n):
            c, ki = kmap[k]
            t = wx_c[c]
            if c == 0 and KCHUNKS[0] == 1:
                xbase = 0          # chunk 0 is [X_k0 | W_k0]
                wbase = C + n * NTILE
            else:
                xbase = ki * Q + F
                wbase = ki * Q + n * NTILE
            nc.tensor.matmul(
                ps[m][:, n * NTILE:(n + 1) * NTILE],
                lhsT=t[:, xbase + m * P:xbase + (m + 1) * P],
                rhs=t[:, wbase:wbase + NTILE],
                start=(k == 0),
                stop=(k == KT - 1),
            )

        # k0 runs n-major so its first four matmuls need only chunk 0a.
        for n in range(NT):
            for m in range(MT):
                mm(m, 0, n)
        for k in range(1, PHASE1_K):
            for m in range(MT):
                for n in range(NT):
                    mm(m, k, n)
        # Phase 2: finish each m-tile (k-inner so the n0/n1 matmul pairs
        # keep a shared lhsT for the LDWEIGHTS dedup) and evict its two
        # PSUM-bank halves; the n0 copy only waits on the k7/n0 matmul so
        # it overlaps k7/n1. Copy engines and DMA rings alternate
        # scalar/gpsimd+vector so consecutive 256 KB output DMAs overlap.
        for m in range(MT):
            for k in range(PHASE1_K, KT):
                for n in range(NT):
                    mm(m, k, n)
            for n in range(NT):
                i = m * NT + n
                yt = yp.tile([P, NTILE], F32, name=f"yt{i}", tag=f"y{i}")
                src = ps[m][:, n * NTILE:(n + 1) * NTILE]
                if i % 2 == 0:
                    nc.scalar.copy(yt[:], src)
                    nc.scalar.dma_start(
                        y[m * P:(m + 1) * P, n * NTILE:(n + 1) * NTILE], yt[:])
                else:
                    nc.vector.tensor_copy(yt[:], src)
                    nc.gpsimd.dma_start(
                        y[m * P:(m + 1) * P, n * NTILE:(n + 1) * NTILE], yt[:])

        # Cooldown: keep the PE array active past the last real matmul so
        # the HAM clock gate stays at K=8/8 while the output drains and
        # the epilogue event-reset walk (dispatched at the PE clock) runs.
        for _ in range(cooldown):
            nc.tensor.matmul(ps[0][:, :NTILE], lhsT=zt[:, :P], rhs=zt[:],
                             start=True, stop=True)


def _dedup_ldweights(nc):
    """Drop InstLdweights whose stationary operand is identical to the PE
    array's current contents (loaded by the previous InstLdweights with only
    matmuls in between). The PE array state is persistent, so the paired
    matmul reuses the already-loaded weights; this halves the PE instruction
    count for n0/n1 matmul pairs sharing an lhsT and for the warmup/cooldown
    runs, shortening both the stream and the epilogue event-reset walk."""
    def key(i):
        a = i.ins[0]
        return (str(a.memref), a.offset, str(a.ap), str(a.dtype),
                i.perf_mode, i.is_transpose, i.tile_position)

    for blk in nc.main_func.blocks:
        out = []
        last = None
        for i in blk.instructions:
            if getattr(i, "engine", None) != mybir.EngineType.PE:
                out.append(i)
                continue
            if isinstance(i, mybir.InstLdweights):
                si = i.sync_info
                clean = si is None or (not si.on_wait and not si.on_update)
                if clean and last is not None and key(i) == last:
                    continue
                last = key(i) if clean else None
                out.append(i)
            elif isinstance(i, mybir.InstMatmult):
                out.append(i)
            else:
                last = None
                out.append(i)
        blk.instructions[:] = out


LAST_PROFILE = {}


def kernel(inputs, indices, kernel, bias, _trace=False):
    x = np.ascontiguousarray(np.asarray(inputs), dtype=np.float32)
    idx = np.asarray(indices).astype(np.int64)
    wk = np.asarray(kernel, dtype=np.float32)
    bv = np.asarray(bias, dtype=np.float32)

    B, D = x.shape
    H, _, F = wk.shape
    C = CAP

    rows = [np.nonzero(idx == h)[0] for h in range(H)]
    kept = [r[:C] for r in rows]
    over = [r[C:] for r in rows]

    def pack(w16, xt16):
        # fused stream: per k-chunk one [P, kg*(F+C)] block where
        # block[p, ki*(F+C) + 0:F]   = W[(k0+ki)*P + p, :]
        # block[p, ki*(F+C) + F:F+C] = XT[(k0+ki)*P + p, :]
        # except chunk 0 (kg=1), which is column-ordered [XT | W] so the
        # device can gate its first matmuls on just the [XT | W_n0] prefix.
        KTl = w16.shape[0] // P
        fused = np.concatenate(
            [w16.reshape(KTl, P, F), xt16.reshape(KTl, P, C)], axis=2
        )  # [KT, P, F+C]
        parts = []
        k0 = 0
        for c, kg in enumerate(KCHUNKS):
            if c == 0 and kg == 1:
                blk0 = np.concatenate(
                    [xt16.reshape(KTl, P, C)[0],
                     w16.reshape(KTl, P, F)[0]], axis=1)
                parts.append(blk0.reshape(-1))
            else:
                blk = fused[k0:k0 + kg]  # [kg, P, Q]
                parts.append(blk.transpose(1, 0, 2).reshape(-1))
            k0 += kg
        return np.concatenate(parts)

    in_maps = []
    for h in range(H):
        r = kept[h]
        xt = np.zeros((D, C), dtype=np.float16)
        xt[:, :len(r)] = x[r].T
        in_maps.append({"wx": pack(wk[h].astype(np.float16), xt)})

    nc = bacc.Bacc(
        "TRN2", target_bir_lowering=False, debug=False, num_devices=H,
        enable_asserts=False,
    )
    _build(nc, C, D, F)
    _dedup_ldweights(nc)
    nc.compile()

    trace_kwargs = (
        {"trace": True, "trace_cores": list(range(H)), "stitch_traces": False}
        if _trace
        else {}
    )
    res = run_bass_kernel_spmd(nc, in_maps, core_ids=list(range(H)), **trace_kwargs)
    if _trace:
        LAST_PROFILE.clear()
        LAST_PROFILE.update(
            exec_time_ns=res.exec_time_ns,
            mean_exec_time_ns=res.mean_exec_time_ns,
            max_exec_time_core_id=res.max_exec_time_core_id,
            trace=res.instructions_and_trace[1] if res.instructions_and_trace else None,
            profile_json=res.profile_json,
        )

    out = np.empty((B, F), dtype=np.float32)
    for h in range(H):
        r = kept[h]
        out[r] = res.results[h]["y"][:len(r)] + bv[h]
        if len(over[h]):
            out[over[h]] = x[over[h]] @ wk[h] + bv[h]
    return out


# revision 23
# speedup vs baseline: 1.0509x; 1.0446x over previous
"""MoE routed dense layer (nn_MultiHeadDense): y[b] = x[b] @ W[idx[b]] + bias[idx[b]].

Full shapes: inputs [4096,1024] f32, indices [4096] int, kernel [8,1024,1024] f32,
bias [8,1024] f32 -> out [4096,1024] f32.

Sharding strategy (expert-parallel, H == n_cores == 8): core h owns expert h's
weight [1024,1024] and processes up to C=512 of the rows routed to expert h.
The host computes the per-expert row lists from `indices`, gathers each
expert's first C rows into a zero-padded transposed activation block
XT_h [D, C], and scatters the per-core outputs back into the full [B, F]
result. Rows beyond C on an overloaded expert (~1% of rows for balanced
routing) are computed on the host in f32; this keeps the device at exactly
4 full 128-row m-tiles (64 matmuls) instead of 5 mostly-empty ones.

On-device per core: Y[c, f] = sum_k XT[k*128:(k+1)*128, c].T @ W[k*128:.., f]
accumulated in PSUM over the 8 k-tiles. The bias is added on the host during
the scatter (exact f32), so the device eviction is a plain PSUM->SBUF copy.
X and W are pre-cast to fp16 on the host (11-bit mantissa keeps the absmax
error ~1e-3 of output scale while halving HBM traffic and enabling the fast
PE weight-load path); accumulation stays fp32 in PSUM.

Schedule: the W+X stream arrives as 6 chunks on the sync HWDGE ring. Phase 1
runs k0..k3 for all four m-tiles k-outermost, racing the DMA fill; phase 2
finishes k4..k7 one (m, n) PSUM-bank half at a time, evicting each 256 KB
half (copy + DMA, engines/rings alternating scalar/vector) as soon as its
accumulation completes, so the output overlaps the remaining matmuls and the
last half leaves right after the final matmul. Zero-matmul warmup bridges PE
activity from queue start until chunk 0 lands so the HAM full-duty window is
granted right as the real stream starts; trailing zero-matmuls keep the PE
clock un-gated (K=8/8) through the backend's end-of-program event-reset walk,
which dispatches on the PE clock and would otherwise run at half rate.
"""

from contextlib import ExitStack

import numpy as np

import concourse.bass as bass
import concourse.tile as tile
from concourse import bacc, mybir
from concourse.bass_utils import run_bass_kernel_spmd

F32 = mybir.dt.float32
F16 = mybir.dt.float16

P = 128          # SBUF partitions / matmul tile edge
NTILE = 512      # matmul moving free dim (one fp32 PSUM bank)
CAP = 512        # device rows per core; overflow rows computed on host
WARMUP_MM = 6    # zero-matmuls bridging PE idle until chunk 0a lands
COOLDOWN_MM = 10  # zero-matmuls holding K=8/8 through the output drain
KCHUNKS = (1, 1, 1, 1, 2, 2)   # k-tiles per input-stream chunk
PHASE1_K = 6     # k0..PHASE1_K-1 run k-outer; the rest run m-outer + evict


def _build(nc: bass.Bass, C: int, D: int, F: int,
           warmup=WARMUP_MM, cooldown=COOLDOWN_MM):
    KT = D // P
    NT = F // NTILE
    MT = C // P
    assert C % P == 0 and sum(KCHUNKS) == KT
    Q = F + C        # columns per k-tile in the fused stream

    wx = nc.dram_tensor("wx", (KT * P * Q,), F16, kind="ExternalInput").ap()
    y = nc.dram_tensor("y", (C, F), F32, kind="ExternalOutput").ap()
    scratch = nc.dram_tensor("scratch", (P, 16), F16, kind="ExternalOutput").ap()

    with tile.TileContext(nc) as tc, ExitStack() as ctx:
        cp = ctx.enter_context(tc.tile_pool(name="cp", bufs=1))
        zp = ctx.enter_context(tc.tile_pool(name="zp", bufs=1))
        pp = ctx.enter_context(tc.tile_pool(name="pp", bufs=4, space="PSUM"))
        yp = ctx.enter_context(tc.tile_pool(name="yp", bufs=8))

        # Input stream (W+X chunks) on the sync HWDGE ring. Chunk 0 is
        # column-ordered [X_k0 | W_k0] and delivered as two DMAs so the
        # k0/n0 matmuls (the first real work) gate on only the leading
        # [X | W_n0] 2/3 of the chunk.
        wx_c = []
        off = 0
        for c, kg in enumerate(KCHUNKS):
            q = kg * Q
            ct = cp.tile([P, q], F16, name=f"wx{c}", tag=f"wx{c}")
            src = wx[off:off + P * q].rearrange("(p q) -> p q", p=P)
            if c == 0 and kg == 1:
                split = C + NTILE
                nc.sync.dma_start(ct[:, :split], src[:, :split])
                nc.sync.dma_start(ct[:, split:], src[:, split:])
            else:
                nc.sync.dma_start(ct[:], src)
            wx_c.append(ct)
            off += P * q

        ps = [pp.tile([P, F], F32, name=f"ps{m}", tag="ps") for m in range(MT)]

        # PE warmup: zero matmuls (no DMA dependency) keep the PE busy
        # until chunk 0's completion receipt lands, so the HAM clock-gate
        # warmup (sustained activity before the PE runs at 2.4 GHz)
        # overlaps the DMA fill instead of following it. They target
        # ps[0], which the first real k=0 matmul resets via start=True.
        zt = zp.tile([P, NTILE], F16)
        nc.vector.memset(zt[:], 0.0)
        # Warm the scalar output DMA ring with a tiny transfer so the
        # evictions later don't pay the ~0.8 us first-DMA ring-startup
        # latency on the kernel's critical tail. (The sync ring, which
        # carries the other half of the evictions, is warmed by the input
        # stream itself. The gpsimd ring is SWDGE — software descriptor
        # generation, ~60-125 GB/s — and is not used.)
        nc.scalar.dma_start(scratch[:, :8], zt[:, :8])
        for _ in range(warmup):
            nc.tensor.matmul(ps[0][:, :NTILE], lhsT=zt[:, :P], rhs=zt[:],
                             start=True, stop=True)

        kmap = []  # k -> (chunk, index within chunk)
        for c, kg in enumerate(KCHUNKS):
            kmap.extend((c, ki) for ki in range(kg))

        def mm(m, k, n):
            c, ki = kmap[k]
            t = wx_c[c]
            if c == 0 and KCHUNKS[0] == 1:
                xbase = 0          # chunk 0 is [X_k0 | W_k0]
                wbase = C + n * NTILE
            else:
                xbase = ki * Q + F
                wbase = ki * Q + n * NTILE
            nc.tensor.matmul(
                ps[m][:, n * NTILE:(n + 1) * NTILE],
                lhsT=t[:, xbase + m * P:xbase + (m + 1) * P],
                rhs=t[:, wbase:wbase + NTILE],
                start=(k == 0),
                stop=(k == KT - 1),
            )

        # k0 runs n-major so its first four matmuls need only chunk 0a.
        for n in range(NT):
            for m in range(MT):
                mm(m, 0, n)
        for k in range(1, PHASE1_K):
            for m in range(MT):
                for n in range(NT):
                    mm(m, k, n)
        # Phase 2: finish each m-tile (k-inner so the n0/n1 matmul pairs
        # keep a shared lhsT for the LDWEIGHTS dedup) and evict its two
        # PSUM-bank halves. The n0 half goes scalar-copy -> scalar ring;
        # the n1 half goes DVE-copy -> sync ring (idle once the input
        # stream is done, which precedes all of phase 2 since PHASE1_K=6
        # leaves only k6/k7 — the last input chunk — for phase 2). Two
        # HWDGE rings drain the 8 x 256 KB output pieces in parallel.
        for m in range(MT):
            for k in range(PHASE1_K, KT):
                for n in range(NT):
                    mm(m, k, n)
            for n in range(NT):
                i = m * NT + n
                yt = yp.tile([P, NTILE], F32, name=f"yt{i}", tag=f"y{i}")
                src = ps[m][:, n * NTILE:(n + 1) * NTILE]
                if n == 0:
                    nc.scalar.copy(yt[:], src)
                    nc.scalar.dma_start(
                        y[m * P:(m + 1) * P, n * NTILE:(n + 1) * NTILE], yt[:])
                else:
                    nc.vector.tensor_copy(yt[:], src)
                    nc.sync.dma_start(
                        y[m * P:(m + 1) * P, n * NTILE:(n + 1) * NTILE], yt[:])

        # Cooldown: keep the PE array active past the last real matmul so
        # the HAM clock gate stays at K=8/8 while the output drains and
        # the epilogue event-reset walk (dispatched at the PE clock) runs.
        for _ in range(cooldown):
            nc.tensor.matmul(ps[0][:, :NTILE], lhsT=zt[:, :P], rhs=zt[:],
                             start=True, stop=True)


def _dedup_ldweights(nc):
    """Drop InstLdweights whose stationary operand is identical to the PE
    array's current contents (loaded by the previous InstLdweights with only
    matmuls in between). The PE array state is persistent, so the paired
    matmul reuses the already-loaded weights; this halves the PE instruction
    count for n0/n1 matmul pairs sharing an lhsT and for the warmup/cooldown
    runs, shortening both the stream and the epilogue event-reset walk."""
    def key(i):
        a = i.ins[0]
        return (str(a.memref), a.offset, str(a.ap), str(a.dtype),
                i.perf_mode, i.is_transpose, i.tile_position)

    for blk in nc.main_func.blocks:
        out = []
        last = None
        for i in blk.instructions:
            if getattr(i, "engine", None) != mybir.EngineType.PE:
                out.append(i)
                continue
            if isinstance(i, mybir.InstLdweights):
                si = i.sync_info
                clean = si is None or (not si.on_wait and not si.on_update)
                if clean and last is not None and key(i) == last:
                    continue
                last = key(i) if clean else None
                out.append(i)
            elif isinstance(i, mybir.InstMatmult):
                out.append(i)
            else:
                last = None
                out.append(i)
        blk.instructions[:] = out


LAST_PROFILE = {}


def kernel(inputs, indices, kernel, bias, _trace=False):
    x = np.ascontiguousarray(np.asarray(inputs), dtype=np.float32)
    idx = np.asarray(indices).astype(np.int64)
    wk = np.asarray(kernel, dtype=np.float32)
    bv = np.asarray(bias, dtype=np.float32)

    B, D = x.shape
    H, _, F = wk.shape
    C = CAP

    rows = [np.nonzero(idx == h)[0] for h in range(H)]
    kept = [r[:C] for r in rows]
    over = [r[C:] for r in rows]

    def pack(w16, xt16):
        # fused stream: per k-chunk one [P, kg*(F+C)] block where
        # block[p, ki*(F+C) + 0:F]   = W[(k0+ki)*P + p, :]
        # block[p, ki*(F+C) + F:F+C] = XT[(k0+ki)*P + p, :]
        # except chunk 0 (kg=1), which is column-ordered [XT | W] so the
        # device can gate its first matmuls on just the [XT | W_n0] prefix.
        KTl = w16.shape[0] // P
        fused = np.concatenate(
            [w16.reshape(KTl, P, F), xt16.reshape(KTl, P, C)], axis=2
        )  # [KT, P, F+C]
        parts = []
        k0 = 0
        for c, kg in enumerate(KCHUNKS):
            if c == 0 and kg == 1:
                blk0 = np.concatenate(
                    [xt16.reshape(KTl, P, C)[0],
                     w16.reshape(KTl, P, F)[0]], axis=1)
                parts.append(blk0.reshape(-1))
            else:
                blk = fused[k0:k0 + kg]  # [kg, P, Q]
                parts.append(blk.transpose(1, 0, 2).reshape(-1))
            k0 += kg
        return np.concatenate(parts)

    in_maps = []
    for h in range(H):
        r = kept[h]
        xt = np.zeros((D, C), dtype=np.float16)
        xt[:, :len(r)] = x[r].T
        in_maps.append({"wx": pack(wk[h].astype(np.float16), xt)})

    nc = bacc.Bacc(
        "TRN2", target_bir_lowering=False, debug=False, num_devices=H,
        enable_asserts=False,
    )
    _build(nc, C, D, F)
    _dedup_ldweights(nc)
    nc.compile()

    trace_kwargs = (
        {"trace": True, "trace_cores": list(range(H)), "stitch_traces": False}
        if _trace
        else {}
    )
    res = run_bass_kernel_spmd(nc, in_maps, core_ids=list(range(H)), **trace_kwargs)
    if _trace:
        LAST_PROFILE.clear()
        LAST_PROFILE.update(
            exec_time_ns=res.exec_time_ns,
            mean_exec_time_ns=res.mean_exec_time_ns,
            max_exec_time_core_id=res.max_exec_time_core_id,
            trace=res.instructions_and_trace[1] if res.instructions_and_trace else None,
            profile_json=res.profile_json,
        )

    out = np.empty((B, F), dtype=np.float32)
    for h in range(H):
        r = kept[h]
        out[r] = res.results[h]["y"][:len(r)] + bv[h]
        if len(over[h]):
            out[over[h]] = x[over[h]] @ wk[h] + bv[h]
    return out


# revision 25
# speedup vs baseline: 1.0605x; 1.0091x over previous
"""MoE routed dense layer (nn_MultiHeadDense): y[b] = x[b] @ W[idx[b]] + bias[idx[b]].

Full shapes: inputs [4096,1024] f32, indices [4096] int, kernel [8,1024,1024] f32,
bias [8,1024] f32 -> out [4096,1024] f32.

Sharding strategy (expert-parallel, H == n_cores == 8): core h owns expert h's
weight [1024,1024] and processes up to C=512 of the rows routed to expert h.
The host computes the per-expert row lists from `indices`, gathers each
expert's first C rows into a zero-padded transposed activation block
XT_h [D, C], and scatters the per-core outputs back into the full [B, F]
result. Rows beyond C on an overloaded expert (~1% of rows for balanced
routing) are computed on the host in f32; this keeps the device at exactly
4 full 128-row m-tiles (64 matmuls) instead of 5 mostly-empty ones.

On-device per core: Y[c, f] = sum_k XT[k*128:(k+1)*128, c].T @ W[k*128:.., f]
accumulated in PSUM over the 8 k-tiles. The bias is added on the host during
the scatter (exact f32), so the device eviction is a plain PSUM->SBUF copy.
X and W are pre-cast to fp16 on the host (11-bit mantissa keeps the absmax
error ~1e-3 of output scale while halving HBM traffic and enabling the fast
PE weight-load path); accumulation stays fp32 in PSUM.

Schedule: the W+X stream arrives as 6 chunks on the sync HWDGE ring. Phase 1
runs k0..k3 for all four m-tiles k-outermost, racing the DMA fill; phase 2
finishes k4..k7 one (m, n) PSUM-bank half at a time, evicting each 256 KB
half (copy + DMA, engines/rings alternating scalar/vector) as soon as its
accumulation completes, so the output overlaps the remaining matmuls and the
last half leaves right after the final matmul. Zero-matmul warmup bridges PE
activity from queue start until chunk 0 lands so the HAM full-duty window is
granted right as the real stream starts; trailing zero-matmuls keep the PE
clock un-gated (K=8/8) through the backend's end-of-program event-reset walk,
which dispatches on the PE clock and would otherwise run at half rate.
"""

from contextlib import ExitStack

import numpy as np

import concourse.bass as bass
import concourse.tile as tile
from concourse import bacc, mybir
from concourse.bass_utils import run_bass_kernel_spmd

F32 = mybir.dt.float32
F16 = mybir.dt.float16

P = 128          # SBUF partitions / matmul tile edge
NTILE = 512      # matmul moving free dim (one fp32 PSUM bank)
CAP = 512        # device rows per core; overflow rows computed on host
WARMUP_MM = 6    # zero-matmuls bridging PE idle until chunk 0a lands
COOLDOWN_MM = 12  # zero-matmuls holding K=8/8 through the output drain
KCHUNKS = (1, 1, 1, 1, 2, 2)   # k-tiles per input-stream chunk
PHASE1_K = 6     # k0..PHASE1_K-1 run k-outer; the rest run m-outer + evict


def _build(nc: bass.Bass, C: int, D: int, F: int,
           warmup=WARMUP_MM, cooldown=COOLDOWN_MM):
    KT = D // P
    NT = F // NTILE
    MT = C // P
    assert C % P == 0 and sum(KCHUNKS) == KT
    Q = F + C        # columns per k-tile in the fused stream

    wx = nc.dram_tensor("wx", (KT * P * Q,), F16, kind="ExternalInput").ap()
    y = nc.dram_tensor("y", (C, F), F32, kind="ExternalOutput").ap()
    scratch = nc.dram_tensor("scratch", (P, 16), F16, kind="ExternalOutput").ap()

    with tile.TileContext(nc) as tc, ExitStack() as ctx:
        cp = ctx.enter_context(tc.tile_pool(name="cp", bufs=1))
        zp = ctx.enter_context(tc.tile_pool(name="zp", bufs=1))
        pp = ctx.enter_context(tc.tile_pool(name="pp", bufs=4, space="PSUM"))
        yp = ctx.enter_context(tc.tile_pool(name="yp", bufs=8))

        # Input stream (W+X chunks) on the sync HWDGE ring. Chunk 0 is
        # column-ordered [X_k0 | W_k0] and delivered as two DMAs so the
        # k0/n0 matmuls (the first real work) gate on only the leading
        # [X | W_n0] 2/3 of the chunk.
        wx_c = []
        off = 0
        for c, kg in enumerate(KCHUNKS):
            q = kg * Q
            ct = cp.tile([P, q], F16, name=f"wx{c}", tag=f"wx{c}")
            src = wx[off:off + P * q].rearrange("(p q) -> p q", p=P)
            if c == 0 and kg == 1:
                split = C + NTILE
                nc.sync.dma_start(ct[:, :split], src[:, :split])
                nc.sync.dma_start(ct[:, split:], src[:, split:])
            else:
                nc.sync.dma_start(ct[:], src)
            wx_c.append(ct)
            off += P * q

        ps = [pp.tile([P, F], F32, name=f"ps{m}", tag="ps") for m in range(MT)]

        # PE warmup: zero matmuls (no DMA dependency) keep the PE busy
        # until chunk 0's completion receipt lands, so the HAM clock-gate
        # warmup (sustained activity before the PE runs at 2.4 GHz)
        # overlaps the DMA fill instead of following it. They target
        # ps[0], which the first real k=0 matmul resets via start=True.
        zt = zp.tile([P, NTILE], F16)
        nc.vector.memset(zt[:], 0.0)
        # Warm the scalar output DMA ring with a tiny transfer so the
        # evictions later don't pay the ~0.8 us first-DMA ring-startup
        # latency on the kernel's critical tail. (The sync ring, which
        # carries the other half of the evictions, is warmed by the input
        # stream itself. The gpsimd ring is SWDGE — software descriptor
        # generation, ~60-125 GB/s — and is not used.)
        nc.scalar.dma_start(scratch[:, :8], zt[:, :8])
        for _ in range(warmup):
            nc.tensor.matmul(ps[0][:, :NTILE], lhsT=zt[:, :P], rhs=zt[:],
                             start=True, stop=True)

        kmap = []  # k -> (chunk, index within chunk)
        for c, kg in enumerate(KCHUNKS):
            kmap.extend((c, ki) for ki in range(kg))

        def mm(m, k, n):
            c, ki = kmap[k]
            t = wx_c[c]
            if c == 0 and KCHUNKS[0] == 1:
                xbase = 0          # chunk 0 is [X_k0 | W_k0]
                wbase = C + n * NTILE
            else:
                xbase = ki * Q + F
                wbase = ki * Q + n * NTILE
            nc.tensor.matmul(
                ps[m][:, n * NTILE:(n + 1) * NTILE],
                lhsT=t[:, xbase + m * P:xbase + (m + 1) * P],
                rhs=t[:, wbase:wbase + NTILE],
                start=(k == 0),
                stop=(k == KT - 1),
            )

        # k0 runs n-major so its first four matmuls need only chunk 0a.
        for n in range(NT):
            for m in range(MT):
                mm(m, 0, n)
        for k in range(1, PHASE1_K):
            for m in range(MT):
                for n in range(NT):
                    mm(m, k, n)
        # Phase 2: finish each m-tile (k-inner so the n0/n1 matmul pairs
        # keep a shared lhsT for the LDWEIGHTS dedup) and evict it. Both
        # PSUM-bank halves are copied in parallel (scalar engine takes n0,
        # DVE takes n1) into one [P, F] tile that leaves as a single
        # 512 KB DMA with 4 KB per-partition lines; the rings alternate
        # scalar/sync (the sync ring is idle once the input stream is
        # done, which precedes all of phase 2 since PHASE1_K=6 leaves
        # only k6/k7 — the last input chunk — for phase 2). The last
        # m-tile instead leaves as two 256 KB half-DMAs, one per ring, so
        # the kernel tail is half a DMA rather than a full one.
        for m in range(MT):
            for k in range(PHASE1_K, KT):
                for n in range(NT):
                    mm(m, k, n)
            yt = yp.tile([P, F], F32, name=f"yt{m}", tag=f"y{m}")
            nc.scalar.copy(yt[:, :NTILE], ps[m][:, :NTILE])
            nc.vector.tensor_copy(yt[:, NTILE:], ps[m][:, NTILE:])
            rows = y[m * P:(m + 1) * P, :]
            if m < MT - 1:
                eng = nc.scalar if m % 2 == 0 else nc.sync
                eng.dma_start(rows, yt[:])
            else:
                nc.sync.dma_start(rows[:, :NTILE], yt[:, :NTILE])
                nc.scalar.dma_start(rows[:, NTILE:], yt[:, NTILE:])

        # Cooldown: keep the PE array active past the last real matmul so
        # the HAM clock gate stays at K=8/8 while the output drains and
        # the epilogue event-reset walk (dispatched at the PE clock) runs.
        for _ in range(cooldown):
            nc.tensor.matmul(ps[0][:, :NTILE], lhsT=zt[:, :P], rhs=zt[:],
                             start=True, stop=True)


def _dedup_ldweights(nc):
    """Drop InstLdweights whose stationary operand is identical to the PE
    array's current contents (loaded by the previous InstLdweights with only
    matmuls in between). The PE array state is persistent, so the paired
    matmul reuses the already-loaded weights; this halves the PE instruction
    count for n0/n1 matmul pairs sharing an lhsT and for the warmup/cooldown
    runs, shortening both the stream and the epilogue event-reset walk."""
    def key(i):
        a = i.ins[0]
        return (str(a.memref), a.offset, str(a.ap), str(a.dtype),
                i.perf_mode, i.is_transpose, i.tile_position)

    for blk in nc.main_func.blocks:
        out = []
        last = None
        for i in blk.instructions:
            if getattr(i, "engine", None) != mybir.EngineType.PE:
                out.append(i)
                continue
            if isinstance(i, mybir.InstLdweights):
                si = i.sync_info
                clean = si is None or (not si.on_wait and not si.on_update)
                if clean and last is not None and key(i) == last:
                    continue
                last = key(i) if clean else None
                out.append(i)
            elif isinstance(i, mybir.InstMatmult):
                out.append(i)
            else:
                last = None
                out.append(i)
        blk.instructions[:] = out


LAST_PROFILE = {}


def kernel(inputs, indices, kernel, bias, _trace=False):
    x = np.ascontiguousarray(np.asarray(inputs), dtype=np.float32)
    idx = np.asarray(indices).astype(np.int64)
    wk = np.asarray(kernel, dtype=np.float32)
    bv = np.asarray(bias, dtype=np.float32)

    B, D = x.shape
    H, _, F = wk.shape
    C = CAP

    rows = [np.nonzero(idx == h)[0] for h in range(H)]
    kept = [r[:C] for r in rows]
    over = [r[C:] for r in rows]

    def pack(w16, xt16):
        # fused stream: per k-chunk one [P, kg*(F+C)] block where
        # block[p, ki*(F+C) + 0:F]   = W[(k0+ki)*P + p, :]
        # block[p, ki*(F+C) + F:F+C] = XT[(k0+ki)*P + p, :]
        # except chunk 0 (kg=1), which is column-ordered [XT | W] so the
        # device can gate its first matmuls on just the [XT | W_n0] prefix.
        KTl = w16.shape[0] // P
        fused = np.concatenate(
            [w16.reshape(KTl, P, F), xt16.reshape(KTl, P, C)], axis=2
        )  # [KT, P, F+C]
        parts = []
        k0 = 0
        for c, kg in enumerate(KCHUNKS):
            if c == 0 and kg == 1:
                blk0 = np.concatenate(
                    [xt16.reshape(KTl, P, C)[0],
                     w16.reshape(KTl, P, F)[0]], axis=1)
                parts.append(blk0.reshape(-1))
            else:
                blk = fused[k0:k0 + kg]  # [kg, P, Q]
                parts.append(blk.transpose(1, 0, 2).reshape(-1))
            k0 += kg
        return np.concatenate(parts)

    in_maps = []
    for h in range(H):
        r = kept[h]
        xt = np.zeros((D, C), dtype=np.float16)
        xt[:, :len(r)] = x[r].T
        in_maps.append({"wx": pack(wk[h].astype(np.float16), xt)})

    nc = bacc.Bacc(
        "TRN2", target_bir_lowering=False, debug=False, num_devices=H,
        enable_asserts=False,
    )
    _build(nc, C, D, F)
    _dedup_ldweights(nc)
    nc.compile()

    trace_kwargs = (
        {"trace": True, "trace_cores": list(range(H)), "stitch_traces": False}
        if _trace
        else {}
    )
    res = run_bass_kernel_spmd(nc, in_maps, core_ids=list(range(H)), **trace_kwargs)
    if _trace:
        LAST_PROFILE.clear()
        LAST_PROFILE.update(
            exec_time_ns=res.exec_time_ns,
            mean_exec_time_ns=res.mean_exec_time_ns,
            max_exec_time_core_id=res.max_exec_time_core_id,
            trace=res.instructions_and_trace[1] if res.instructions_and_trace else None,
            profile_json=res.profile_json,
        )

    out = np.empty((B, F), dtype=np.float32)
    for h in range(H):
        r = kept[h]
        out[r] = res.results[h]["y"][:len(r)] + bv[h]
        if len(over[h]):
            out[over[h]] = x[over[h]] @ wk[h] + bv[h]
    return out


# revision 26
# speedup vs baseline: 1.1296x; 1.0652x over previous
"""MoE routed dense layer (nn_MultiHeadDense): y[b] = x[b] @ W[idx[b]] + bias[idx[b]].

Full shapes: inputs [4096,1024] f32, indices [4096] int, kernel [8,1024,1024] f32,
bias [8,1024] f32 -> out [4096,1024] f32.

Sharding strategy (expert-parallel, H == n_cores == 8): core h owns expert h's
weight [1024,1024] and processes up to C=512 of the rows routed to expert h.
The host computes the per-expert row lists from `indices`, gathers each
expert's first C rows into a zero-padded transposed activation block
XT_h [D, C], and scatters the per-core outputs back into the full [B, F]
result. Rows beyond C on an overloaded expert (~1% of rows for balanced
routing) are computed on the host in f32; this keeps the device at exactly
4 full 128-row m-tiles (64 matmuls) instead of 5 mostly-empty ones.

On-device per core: Y[c, f] = sum_k XT[k*128:(k+1)*128, c].T @ W[k*128:.., f]
accumulated in PSUM over the 8 k-tiles, bias added during the PSUM->SBUF
eviction. X and W are pre-cast to fp16 on the host (11-bit mantissa keeps the
absmax error ~1e-3 of output scale while halving HBM traffic and enabling the
fast PE weight-load path); accumulation stays fp32 in PSUM and bias is added
in fp32.

Schedule: the W+X stream arrives as 6 chunks on the sync HWDGE ring while the
bias rides the otherwise-idle scalar (output) ring. Phase 1 runs k0..k3 for
all four m-tiles k-outermost, racing the DMA fill; phase 2 runs k4..k7
m-outermost so each m-tile's eviction + 512 KB output DMA starts ~2 us apart
and overlaps the remaining matmuls instead of bunching after the stream.
Zero-matmul warmup bridges PE activity from queue start until chunk 0 lands
so the HAM full-duty window is granted as early as possible.
"""

from contextlib import ExitStack

import numpy as np

import concourse.bass as bass
import concourse.tile as tile
from concourse import bacc, mybir
from concourse.bass_utils import run_bass_kernel_spmd

F32 = mybir.dt.float32
F16 = mybir.dt.float16

P = 128          # SBUF partitions / matmul tile edge
NTILE = 512      # matmul moving free dim (one fp32 PSUM bank)
CAP = 512        # device rows per core; overflow rows computed on host
WARMUP_MM = 8    # zero-matmuls bridging PE idle until chunk 0 lands
KCHUNKS = (1, 1, 1, 1, 2, 2)   # k-tiles per input-stream chunk
PHASE1_K = 4     # k0..PHASE1_K-1 run k-outer; the rest run m-outer + evict


def _build(nc: bass.Bass, C: int, D: int, F: int, warmup=WARMUP_MM):
    KT = D // P
    NT = F // NTILE
    MT = C // P
    assert C % P == 0 and sum(KCHUNKS) == KT
    Q = F + C        # columns per k-tile in the fused stream

    wx = nc.dram_tensor("wx", (KT * P * Q,), F16, kind="ExternalInput").ap()
    bias_d = nc.dram_tensor("bias", (P * F,), F16, kind="ExternalInput").ap()
    y = nc.dram_tensor("y", (C, F), F32, kind="ExternalOutput").ap()

    with tile.TileContext(nc) as tc, ExitStack() as ctx:
        cp = ctx.enter_context(tc.tile_pool(name="cp", bufs=1))
        zp = ctx.enter_context(tc.tile_pool(name="zp", bufs=1))
        pp = ctx.enter_context(tc.tile_pool(name="pp", bufs=4, space="PSUM"))
        yp = ctx.enter_context(tc.tile_pool(name="yp", bufs=4))

        # Input stream (W+X chunks) on the sync HWDGE ring; bias on the
        # scalar ring, which otherwise idles until the output DMAs start.
        bias_t = cp.tile([P, F], F16, name="bias", tag="bias")
        nc.scalar.dma_start(
            bias_t[:], bias_d[:].rearrange("(p q) -> p q", p=P))
        wx_c = []
        off = 0
        for c, kg in enumerate(KCHUNKS):
            q = kg * Q
            ct = cp.tile([P, q], F16, name=f"wx{c}", tag=f"wx{c}")
            nc.sync.dma_start(
                ct[:], wx[off:off + P * q].rearrange("(p q) -> p q", p=P))
            wx_c.append(ct)
            off += P * q

        ps = [pp.tile([P, F], F32, name=f"ps{m}", tag="ps") for m in range(MT)]

        # PE warmup: zero matmuls (no DMA dependency) keep the PE busy
        # until chunk 0's completion receipt lands, so the HAM clock-gate
        # warmup (sustained activity before the PE runs at 2.4 GHz)
        # overlaps the DMA fill instead of following it. They target
        # ps[0], which the first real k=0 matmul resets via start=True.
        zt = zp.tile([P, NTILE], F16)
        nc.vector.memset(zt[:], 0.0)
        for _ in range(warmup):
            nc.tensor.matmul(ps[0][:, :NTILE], lhsT=zt[:, :P], rhs=zt[:],
                             start=True, stop=True)

        kmap = []  # k -> (chunk, index within chunk)
        for c, kg in enumerate(KCHUNKS):
            kmap.extend((c, ki) for ki in range(kg))

        def mm(m, k, n):
            c, ki = kmap[k]
            t = wx_c[c]
            xbase = ki * Q + F
            wbase = ki * Q + n * NTILE
            nc.tensor.matmul(
                ps[m][:, n * NTILE:(n + 1) * NTILE],
                lhsT=t[:, xbase + m * P:xbase + (m + 1) * P],
                rhs=t[:, wbase:wbase + NTILE],
                start=(k == 0),
                stop=(k == KT - 1),
            )

        for k in range(PHASE1_K):
            for m in range(MT):
                for n in range(NT):
                    mm(m, k, n)
        for m in range(MT):
            for k in range(PHASE1_K, KT):
                for n in range(NT):
                    mm(m, k, n)
            yt = yp.tile([P, F], F32, name=f"yt{m}", tag="y")
            nc.vector.tensor_add(yt[:], ps[m][:], bias_t[:])
            nc.scalar.dma_start(y[m * P:(m + 1) * P, :], yt[:])


LAST_PROFILE = {}


def kernel(inputs, indices, kernel, bias, _trace=False):
    x = np.ascontiguousarray(np.asarray(inputs), dtype=np.float32)
    idx = np.asarray(indices).astype(np.int64)
    wk = np.asarray(kernel, dtype=np.float32)
    bv = np.asarray(bias, dtype=np.float32)

    B, D = x.shape
    H, _, F = wk.shape
    C = CAP

    rows = [np.nonzero(idx == h)[0] for h in range(H)]
    kept = [r[:C] for r in rows]
    over = [r[C:] for r in rows]

    def pack(w16, xt16):
        # fused stream: per k-chunk one [P, kg*(F+C)] block where
        # block[p, ki*(F+C) + 0:F]   = W[(k0+ki)*P + p, :]
        # block[p, ki*(F+C) + F:F+C] = XT[(k0+ki)*P + p, :]
        KTl = w16.shape[0] // P
        fused = np.concatenate(
            [w16.reshape(KTl, P, F), xt16.reshape(KTl, P, C)], axis=2
        )  # [KT, P, F+C]
        parts = []
        k0 = 0
        for kg in KCHUNKS:
            blk = fused[k0:k0 + kg]  # [kg, P, Q]
            parts.append(blk.transpose(1, 0, 2).reshape(-1))
            k0 += kg
        return np.concatenate(parts)

    in_maps = []
    for h in range(H):
        r = kept[h]
        xt = np.zeros((D, C), dtype=np.float16)
        xt[:, :len(r)] = x[r].T
        in_maps.append({
            "wx": pack(wk[h].astype(np.float16), xt),
            "bias": np.broadcast_to(bv[h].astype(np.float16), (P, F)).reshape(-1),
        })

    nc = bacc.Bacc(
        "TRN2", target_bir_lowering=False, debug=False, num_devices=H,
        enable_asserts=False,
    )
    _build(nc, C, D, F)
    nc.compile()

    trace_kwargs = (
        {"trace": True, "trace_cores": list(range(H)), "stitch_traces": False}
        if _trace
        else {}
    )
    res = run_bass_kernel_spmd(nc, in_maps, core_ids=list(range(H)), **trace_kwargs)
    if _trace:
        LAST_PROFILE.clear()
        LAST_PROFILE.update(
            exec_time_ns=res.exec_time_ns,
            mean_exec_time_ns=res.mean_exec_time_ns,
            max_exec_time_core_id=res.max_exec_time_core_id,
            trace=res.instructions_and_trace[1] if res.instructions_and_trace else None,
            profile_json=res.profile_json,
        )

    out = np.empty((B, F), dtype=np.float32)
    for h in range(H):
        r = kept[h]
        out[r] = res.results[h]["y"][:len(r)]
        if len(over[h]):
            out[over[h]] = x[over[h]] @ wk[h] + bv[h]
    return out
